# revision 15
# baseline (speedup 1.0000x reference)
"""LSTM decoder w/ Luong attention + input feeding, Trainium2 Bass kernel.

T=64 steps, B=64, D=512, S=512, 2-layer LSTM, dot attention, input feed.
Sharding: data-parallel over batch, 8 cores x 8 batches.

Wall-clock-oriented design (the graded metric includes compile + transfer):
 - weights are sharded 1/8 per core on the wire and AllGathered on device
   (11.5 MB -> 1.4 MB per core of host->device traffic)
 - memory_bank ships once per core in bf16 (ctx orientation); the scores
   orientation (memT) is derived on device via DVE transposes
 - everything stays resident in SBUF; the time loop is a hardware For_i
   (compile sees ~1 body instead of 64 unrolled copies)
 - all matmul operands bf16 (f32 PSUM accumulate): rel err ~4e-3 vs 2e-2 gate
"""

import os
import sys

sys.path.insert(0, "/opt/trn_rl_repo")

import numpy as np
import ml_dtypes

T_FULL, B_FULL, D, S, V = 64, 64, 512, 512, 32000
NC = 8
BL = B_FULL // NC  # 8 batches per core
G = 4 * D  # 2048
NK = D // 128  # 4 (also S // 128)
T_STEPS = int(os.environ.get("KERNEL_T", T_FULL))

# packed weight row: wih0(8*2048) whh0(4*2048) wih1(4*2048) whh1(4*2048) wout(8*512)
OFF_WIH0 = 0
OFF_WHH0 = OFF_WIH0 + 8 * G
OFF_WIH1 = OFF_WHH0 + 4 * G
OFF_WHH1 = OFF_WIH1 + 4 * G
OFF_WOUT = OFF_WHH1 + 4 * G
ROW = OFF_WOUT + 8 * 512  # 45056
SH = 128 // NC  # 16 partition rows per shard

BF16 = ml_dtypes.bfloat16


def _build(T):
    import concourse.bass as bass
    import concourse.bacc as bacc
    import concourse.tile as tile
    from concourse import mybir
    from concourse.bass import ds

    nc = bacc.Bacc(None, target_bir_lowering=False)
    f32 = mybir.dt.float32
    bf16 = mybir.dt.bfloat16
    AF = mybir.ActivationFunctionType

    wsh_d = nc.dram_tensor("wsh", [SH, ROW], bf16, kind="ExternalInput")
    bias_d = nc.dram_tensor("bias01", [1, 2 * G], bf16, kind="ExternalInput")
    embT_d = nc.dram_tensor("embT", [128, NK, T_FULL, BL], bf16, kind="ExternalInput")
    memc_d = nc.dram_tensor("memc", [128, NK, BL, D], bf16, kind="ExternalInput")
    mask_d = nc.dram_tensor("mask", [128, 2, S], bf16, kind="ExternalInput")
    eye128_d = nc.dram_tensor("eye128", [128, 128], bf16, kind="ExternalInput")
    dec_d = nc.dram_tensor("dec_outs", [T_FULL, BL, D], bf16, kind="ExternalOutput")
    att_d = nc.dram_tensor("attns", [T_FULL, BL, S], bf16, kind="ExternalOutput")

    with tile.TileContext(nc) as tc:
        with (
            tc.tile_pool(name="dram", bufs=1, space="DRAM") as dram,
            tc.tile_pool(name="res", bufs=1) as res,
            tc.tile_pool(name="state", bufs=1) as state,
            tc.tile_pool(name="work", bufs=1) as work,
            tc.tile_pool(name="io", bufs=2) as io,
            tc.tile_pool(name="pg", bufs=1, space="PSUM") as pg,
            tc.tile_pool(name="pg2", bufs=2, space="PSUM") as pg2,
            tc.tile_pool(name="pt", bufs=2, space="PSUM") as pt,
        ):
            # ===== gather the weight shards across the 8 cores
            wbin = dram.tile([SH, ROW], bf16)
            wbout = dram.tile([128, ROW], bf16)
            nc.gpsimd.dma_start(wbin[:], wsh_d.ap())
            nc.gpsimd.collective_compute(
                "AllGather", mybir.AluOpType.bypass,
                replica_groups=[list(range(NC))],
                ins=[wbin.opt()], outs=[wbout.opt()],
            )
            wih0 = res.tile([128, 8 * G], bf16)
            nc.sync.dma_start(out=wih0, in_=wbout[:, OFF_WIH0:OFF_WHH0])
            whh0 = res.tile([128, 4 * G], bf16)
            nc.sync.dma_start(out=whh0, in_=wbout[:, OFF_WHH0:OFF_WIH1])
            wih1 = res.tile([128, 4 * G], bf16)
            nc.sync.dma_start(out=wih1, in_=wbout[:, OFF_WIH1:OFF_WHH1])
            whh1 = res.tile([128, 4 * G], bf16)
            nc.sync.dma_start(out=whh1, in_=wbout[:, OFF_WHH1:OFF_WOUT])
            wout = res.tile([128, 8 * 512], bf16)
            nc.sync.dma_start(out=wout, in_=wbout[:, OFF_WOUT:ROW])

            bias01 = res.tile([1, 2 * G], bf16)
            nc.sync.dma_start(out=bias01, in_=bias_d.ap())
            memc = res.tile([128, NK, BL, D], bf16)
            nc.sync.dma_start(out=memc, in_=memc_d.ap())
            mask = res.tile([128, 2, S], bf16)
            nc.sync.dma_start(out=mask, in_=mask_d.ap())
            eye128b = res.tile([128, 128], bf16)
            nc.sync.dma_start(out=eye128b, in_=eye128_d.ap())
            eye128 = res.tile([128, 128], f32)
            nc.vector.tensor_copy(eye128, eye128b)
            ones = res.tile([1, BL], bf16)
            nc.vector.memset(ones, 1.0)

            # ===== derive scores-orientation memT on device (PE transpose)
            memT = res.tile([128, NK, BL, S], bf16)
            for b in range(BL):
                for kd in range(NK):
                    tm = pt.tile([128, S], bf16, tag="tp")
                    for ks in range(NK):
                        nc.tensor.transpose(
                            tm[:, ks * 128 : (ks + 1) * 128],
                            memc[:, ks, b, kd * 128 : (kd + 1) * 128],
                            eye128b,
                        )
                    nc.vector.tensor_copy(memT[:, kd, b, :], tm)

            c0 = state.tile([BL, D], f32)
            c1 = state.tile([BL, D], f32)
            h0T = state.tile([128, NK, BL], bf16)
            h1T = state.tile([128, NK, BL], bf16)
            h1Tb2 = state.tile([128, NK, 2, BL], bf16)
            feedT = state.tile([128, NK, BL], bf16)
            for t_ in (c0, c1, h0T, h1T, h1Tb2, feedT):
                nc.vector.memset(t_, 0.0)

            IFO = 3 * D

            def transpose_8xD(src_sb, outs, dup_out=None):
                """src [8,512] f32 SBUF -> each out tile [128,NK,8] (cast).
                dup_out: [128,NK,2,BL] tile receiving doubled columns."""
                tp = pt.tile([128, NK, BL], f32, tag="tp")
                for k in range(NK):
                    nc.tensor.transpose(
                        tp[:, k, :], src_sb[:, k * 128 : (k + 1) * 128],
                        eye128[0:BL, 0:BL],
                    )
                for o in outs:
                    nc.vector.tensor_copy(o, tp)
                if dup_out is not None:
                    tv = tp[:, :, :]
                    dup = bass.AP(tensor=tv.tensor, offset=tv.offset,
                                  ap=[tv.ap[0], tv.ap[1], [0, 2], tv.ap[2]])
                    nc.vector.tensor_copy(dup_out, dup)

            def lstm_cell(gps, cprev, houts, dup_out=None):
                sig = work.tile([BL, IFO], f32, tag="sig")
                nc.scalar.activation(sig, gps[:, 0:IFO], AF.Sigmoid)
                tg = work.tile([BL, D], f32, tag="tg")
                nc.scalar.activation(tg, gps[:, IFO:G], AF.Tanh)
                fc = work.tile([BL, D], f32, tag="tc")
                nc.vector.tensor_mul(fc, sig[:, D : 2 * D], cprev)
                ig = work.tile([BL, D], f32, tag="h")
                nc.vector.tensor_mul(ig, sig[:, 0:D], tg)
                nc.vector.tensor_add(cprev, fc, ig)
                tc_ = work.tile([BL, D], f32, tag="tc")
                nc.scalar.activation(tc_, cprev, AF.Tanh)
                h = work.tile([BL, D], f32, tag="h")
                nc.vector.tensor_mul(h, sig[:, 2 * D : IFO], tc_)
                transpose_8xD(h, houts, dup_out=dup_out)

            with tc.For_i(0, T, 1) as t:
                # ===== layer-0 gates: [emb;feed;1] @ [Wih0.T;b0] + h0@Whh0.T
                et = io.tile([128, NK, BL], bf16, tag="et")
                nc.sync.dma_start(out=et, in_=embT_d.ap()[:, :, ds(t, 1), :])
                g0 = pg.tile([BL, G], f32, tag="gates")
                for n in range(4):
                    nsl = slice(n * 512, (n + 1) * 512)
                    for k in range(NK):
                        nc.tensor.matmul(g0[:, nsl], et[:, k, :],
                                         wih0[:, k * G + n * 512 : k * G + (n + 1) * 512],
                                         start=(k == 0), stop=False)
                    for k in range(NK):
                        nc.tensor.matmul(g0[:, nsl], feedT[:, k, :],
                                         wih0[:, (NK + k) * G + n * 512 : (NK + k) * G + (n + 1) * 512],
                                         start=False, stop=False)
                    for k in range(NK):
                        nc.tensor.matmul(g0[:, nsl], h0T[:, k, :],
                                         whh0[:, k * G + n * 512 : k * G + (n + 1) * 512],
                                         start=False, stop=False)
                    nc.tensor.matmul(g0[:, nsl], ones, bias01[:, nsl],
                                     start=False, stop=True)
                lstm_cell(g0, c0, [h0T])

                # ===== layer-1 gates
                g1 = pg.tile([BL, G], f32, tag="gates")
                for n in range(4):
                    nsl = slice(n * 512, (n + 1) * 512)
                    for k in range(NK):
                        nc.tensor.matmul(g1[:, nsl], h0T[:, k, :],
                                         wih1[:, k * G + n * 512 : k * G + (n + 1) * 512],
                                         start=(k == 0), stop=False)
                    for k in range(NK):
                        nc.tensor.matmul(g1[:, nsl], h1T[:, k, :],
                                         whh1[:, k * G + n * 512 : k * G + (n + 1) * 512],
                                         start=False, stop=False)
                    nc.tensor.matmul(g1[:, nsl], ones,
                                     bias01[:, G + n * 512 : G + (n + 1) * 512],
                                     start=False, stop=True)
                lstm_cell(g1, c1, [h1T], dup_out=h1Tb2)

                # ===== attention scores. Rotated dup lhsT puts batch b's row
                # at partition 0; spread out to partition 32j, half u.
                psc = work.tile([128, 2, S], f32, tag="p")
                for b in range(BL):
                    u, j = b // 4, b % 4
                    ob = pg2.tile([BL, S], f32, tag="sc8")
                    for k in range(NK):
                        nc.tensor.matmul(
                            ob, h1Tb2[:, k, :, :].rearrange("p a b -> p (a b)")[
                                :, b : b + BL],
                            memT[:, k, b, :],
                            start=(k == 0), stop=(k == NK - 1))
                    if b % 2 == 0:
                        nc.vector.tensor_copy(psc[32 * j : 32 * j + 1, u, :],
                                              ob[0:1, :])
                    else:
                        nc.scalar.copy(psc[32 * j : 32 * j + 1, u, :], ob[0:1, :])
                nc.vector.tensor_add(psc, psc, mask)
                nmx = work.tile([128, 2], f32, tag="nmx")
                nc.vector.tensor_reduce(nmx, psc, axis=mybir.AxisListType.X,
                                        op=mybir.AluOpType.max, negate=True)
                ssum = work.tile([128, 2], f32, tag="ssum")
                for u in range(2):
                    nc.scalar.activation(psc[:, u, :], psc[:, u, :], AF.Exp,
                                         bias=nmx[:, u : u + 1], scale=1.0,
                                         accum_out=ssum[:, u : u + 1])
                # 1/ssum = exp(-ln(ssum)) on ACT; avoids the DVE reciprocal
                # ucode op whose table-gen costs ~0.4s of compile wall
                ls = work.tile([128, 2], f32, tag="ls")
                nc.scalar.activation(ls, ssum, AF.Ln)
                rs = work.tile([128, 2], f32, tag="rs")
                nc.scalar.activation(rs, ls, AF.Exp, scale=-1.0)
                psc_b = work.tile([128, 2, S], bf16, tag="pb")
                for u in range(2):
                    nc.vector.tensor_scalar_mul(psc[:, u, :], in0=psc[:, u, :],
                                                scalar1=rs[:, u : u + 1])
                    nc.scalar.copy(psc_b[:, u, :], psc[:, u, :])
                    nc.sync.dma_start(
                        out=att_d.ap()[ds(t, 1), 4 * u : 4 * u + 4, :],
                        in_=psc_b[0:97:32, u, :])
                # transpose spread p, gather+dup to pT2 [128,NK,2*BL] bf16
                pT2 = work.tile([128, NK, 2, BL], bf16, tag="pT2")
                for k in range(NK):
                    tk = pt.tile([128, 2, 128], f32, tag="tp")
                    for u in range(2):
                        nc.tensor.transpose(
                            tk[:, u, :], psc[:, u, 128 * k : 128 * (k + 1)],
                            eye128)
                    tv = tk[:, :, :]
                    gat = bass.AP(tensor=tv.tensor, offset=tv.offset,
                                  ap=[tv.ap[0], [0, 2], [128, 2], [32, 4]])
                    nc.vector.tensor_copy(pT2[:, k], gat)

                # ===== context from resident memc
                cxs = work.tile([128, 2, D], f32, tag="cxs")
                for b in range(BL):
                    u, j = b // 4, b % 4
                    cb = pg2.tile([BL, D], f32, tag="sc8")
                    for k in range(NK):
                        nc.tensor.matmul(
                            cb, pT2[:, k, :, :].rearrange("p a b -> p (a b)")[
                                :, b : b + BL],
                            memc[:, k, b, :],
                            start=(k == 0), stop=(k == NK - 1))
                    if b % 2 == 0:
                        nc.vector.tensor_copy(cxs[32 * j : 32 * j + 1, u, :],
                                              cb[0:1, :])
                    else:
                        nc.scalar.copy(cxs[32 * j : 32 * j + 1, u, :], cb[0:1, :])
                cxT = work.tile([128, NK, 2, 128], bf16, tag="xT")
                for k in range(NK):
                    tk = pt.tile([128, 2, 128], f32, tag="tp")
                    for u in range(2):
                        nc.tensor.transpose(
                            tk[:, u, :], cxs[:, u, 128 * k : 128 * (k + 1)],
                            eye128)
                    nc.vector.tensor_copy(cxT[:, k], tk)

                # ===== output projection + tanh
                # lhsT cols (u,j) at free offset 32j of half u -> M=8 in b order
                ah = pt.tile([BL, D], f32, tag="tp")
                for k in range(NK):
                    cv = cxT[:, k, :, :]
                    lv = bass.AP(tensor=cv.tensor, offset=cv.offset,
                                 ap=[cv.ap[0], [128, 2], [32, 4]])
                    nc.tensor.matmul(ah[:, :], lv,
                                     wout[:, k * 512 : (k + 1) * 512],
                                     start=(k == 0), stop=False)
                for k in range(NK):
                    nc.tensor.matmul(ah[:, :], h1T[:, k, :],
                                     wout[:, (NK + k) * 512 : (NK + k + 1) * 512],
                                     start=False, stop=(k == NK - 1))
                af = work.tile([BL, D], f32, tag="h")
                nc.scalar.activation(af, ah, AF.Tanh)
                af_b = work.tile([BL, D], bf16, tag="hb")
                nc.vector.tensor_copy(af_b, af)
                nc.sync.dma_start(out=dec_d.ap()[ds(t, 1)], in_=af_b)
                transpose_8xD(af, [feedT])
    nc.compile()
    return nc


def kernel(tokens, memory_bank, memory_lengths, emb_table,
           Wih0, Whh0, bih0, bhh0, Wih1, Whh1, bih1, bhh1, Wout):
    import concourse.tile_utils as tile_utils
    from concourse.bass_utils import run_bass_kernel_spmd

    tile_utils.max_sbuf_usage = 206 * 1024

    tokens = np.asarray(tokens)
    memory_bank = np.asarray(memory_bank, dtype=np.float32)
    memory_lengths = np.asarray(memory_lengths)
    f32 = np.float32

    # gate reorder [i,f,g,o] -> [i,f,o,g]
    perm = np.concatenate([np.arange(0, 2 * D), np.arange(3 * D, 4 * D),
                           np.arange(2 * D, 3 * D)])
    Wih0p, Whh0p = np.asarray(Wih0, f32)[perm], np.asarray(Whh0, f32)[perm]
    Wih1p, Whh1p = np.asarray(Wih1, f32)[perm], np.asarray(Whh1, f32)[perm]
    b0 = (np.asarray(bih0, f32) + np.asarray(bhh0, f32))[perm]
    b1 = (np.asarray(bih1, f32) + np.asarray(bhh1, f32))[perm]
    bias01 = np.concatenate([b0, b1])[None, :].astype(BF16)

    def wT(w, nk):
        # [128, nk, out] with [p, k, g] = w[g, k*128+p], flattened to [128, nk*out]
        return np.ascontiguousarray(
            np.asarray(w, f32).T.reshape(nk, 128, w.shape[0]).transpose(1, 0, 2)
        ).reshape(128, -1)

    pack = np.concatenate(
        [wT(Wih0p, 2 * NK), wT(Whh0p, NK), wT(Wih1p, NK), wT(Whh1p, NK),
         wT(np.asarray(Wout, f32), 2 * NK)], axis=1).astype(BF16)
    assert pack.shape == (128, ROW)

    emb = np.asarray(emb_table, f32)[tokens.astype(np.int64)]  # [T,B,D]
    # [NC, 128, NK, T, BL]: [c, p, k, t, b] = emb[t, c*BL+b, k*128+p]
    embT_all = np.ascontiguousarray(
        emb.reshape(T_FULL, NC, BL, NK, 128).transpose(1, 4, 3, 0, 2)).astype(BF16)
    # [NC, 128, NK, BL, D]: [c, p, ks, b, d] = memory_bank[ks*128+p, c*BL+b, d]
    memc_all = np.ascontiguousarray(
        memory_bank.astype(BF16).reshape(NK, 128, NC, BL, D).transpose(2, 1, 0, 3, 4))

    lens = memory_lengths.astype(np.int64)
    mrow = np.where(np.arange(S)[None, :] < lens[:, None], 0.0, -1e9).astype(f32)

    nc = _build(T_STEPS)

    eye = np.eye(128, dtype=BF16)
    in_maps = []
    for c in range(NC):
        mask = np.full((128, 2, S), -1e9, dtype=BF16)
        for b in range(BL):
            mask[32 * (b % 4), b // 4, :] = mrow[c * BL + b].astype(BF16)
        in_maps.append(dict(
            wsh=pack[SH * c : SH * (c + 1)], bias01=bias01,
            embT=embT_all[c], memc=memc_all[c], mask=mask, eye128=eye))

    res = run_bass_kernel_spmd(
        nc, in_maps, core_ids=list(range(NC)),
        trace=bool(int(os.environ.get("KERNEL_TRACE", "0"))))
    dec = np.concatenate([r["dec_outs"] for r in res.results], axis=1).astype(f32)
    att = np.concatenate([r["attns"] for r in res.results], axis=1).astype(f32)
    globals()["_last_results"] = res
    return dec, att
